# revision 27
# baseline (speedup 1.0000x reference)
"""Multi-head attention (B=2, S=2048, D=1024, H=16, E=64) on 8 NeuronCores.

Sharding: core c = (batch b, head-group hg) with b = c // 4, hg = c % 4.
Each core projects q/k/v for its batch into its 4 heads, runs dense
attention for those heads over the full sequence, and computes a partial
output projection with its 256 rows of Wo.  The host sums the 4 partials
per batch and adds bo (the TP all-reduce, folded into the gather step).

On-chip layout (everything "T" = feature-on-partitions):
  qhT/khT  [256, 2048]  two SBUF tiles [128, 2048]; head h at rows (h%2)*64
  vh'      [2048, 260]  16 tiles-worth in one [128, 4160] tile; per head a
                        65-wide block [vh | ones-col] - the ones column makes
                        the PV matmul emit the softmax denominator as row 64.
  scoresT  psum [t=128, s-pair 1024]; exp evicted by ACT with scale=1/8.
  biases are folded into the projection matmuls via a K=1 matmul against a
  ones row (weights staged host-side with the bias as row 1024).
"""

import numpy as np

B, S, D, H, E = 2, 2048, 1024, 16, 64
HG = 4            # heads per core
N_CORES = 8
EL = E + 1        # 65: head block width in vh' (values + ones column)
DT = D // 128     # 8 contraction tiles
SC = S // 512     # 4 s-chunks of 512

_NC = None        # cached compiled Bass module

# E_pair: 8 blocks [16, 128]; block (j, sc) broadcasts recip row (2j+m//64)*4+sc
# to output partition m — builds the per-head recip tile for a head-pair column
_EALL = np.zeros((16, 16 * E), np.float32)
for _j in range(2):
    for _sc in range(4):
        for _m in range(128):
            _EALL[(2 * _j + _m // 64) * 4 + _sc, (_j * 4 + _sc) * 128 + _m] = 1.0
_ONES = np.ones((1, 512), np.float32)


def _build():
    import concourse.bass as bass
    import concourse.mybir as mybir
    import concourse.tile as tile
    from concourse import bacc

    FP = mybir.dt.float32
    FPR = mybir.dt.float32r
    BF = mybir.dt.bfloat16
    EXP = mybir.ActivationFunctionType.Exp

    nc = bacc.Bacc("TRN2", target_bir_lowering=False, debug=False, num_devices=1)

    xq = nc.dram_tensor("xq", [D, S], BF, kind="ExternalInput").ap()
    xk = nc.dram_tensor("xk", [D, S], BF, kind="ExternalInput").ap()
    xv = nc.dram_tensor("xv", [D, S], BF, kind="ExternalInput").ap()
    wq = nc.dram_tensor("wq", [D + 1, HG * E], BF, kind="ExternalInput").ap()
    wk = nc.dram_tensor("wk", [D + 1, HG * E], BF, kind="ExternalInput").ap()
    wv = nc.dram_tensor("wv", [D + 1, HG * EL], BF, kind="ExternalInput").ap()
    wo = nc.dram_tensor("wo", [HG * E, D], FPR, kind="ExternalInput").ap()
    eall_d = nc.dram_tensor("eall", [16, 16 * E], FPR, kind="ExternalInput").ap()
    ones_d = nc.dram_tensor("ones", [1, 512], BF, kind="ExternalInput").ap()
    out = nc.dram_tensor("out_partial", [S, D], FP, kind="ExternalOutput").ap()

    with tile.TileContext(nc) as tc:
        with (
            tc.tile_pool(name="consts", bufs=1) as cpool,
            tc.tile_pool(name="resident", bufs=1) as rpool,
            tc.tile_pool(name="xin", bufs=12) as xpool,
            tc.tile_pool(name="exp", bufs=3) as epool,
            tc.tile_pool(name="outev", bufs=4) as opool,
            tc.tile_pool(name="stage", bufs=4) as spool,
        ):
            ones = cpool.tile([1, 512], BF, tag="ones")
            nc.gpsimd.dma_start(ones[:], ones_d[:])

            # E_all[k, r*64+j] = (k == r): broadcasts recip row r via matmul
            e_all = cpool.tile([16, 16 * E], FPR, tag="eall")
            nc.gpsimd.dma_start(e_all[:], eall_d[:])

            wq_sb = cpool.tile([128, DT * 256], BF, tag="wq")
            wk_sb = cpool.tile([128, DT * 256], BF, tag="wk")
            wv_sb = cpool.tile([128, DT * 260], BF, tag="wv")
            wqb = cpool.tile([1, 256], BF, tag="wqb")
            wkb = cpool.tile([1, 256], BF, tag="wkb")
            wvb = cpool.tile([1, 260], BF, tag="wvb")
            for dt in range(DT):
                nc.gpsimd.dma_start(
                    wq_sb[:, dt * 256 : (dt + 1) * 256],
                    wq[dt * 128 : (dt + 1) * 128, :],
                )
                nc.gpsimd.dma_start(
                    wk_sb[:, dt * 256 : (dt + 1) * 256],
                    wk[dt * 128 : (dt + 1) * 128, :],
                )
                nc.gpsimd.dma_start(
                    wv_sb[:, dt * 260 : (dt + 1) * 260],
                    wv[dt * 128 : (dt + 1) * 128, :],
                )
            nc.gpsimd.dma_start(wqb[:], wq[D : D + 1, :])
            nc.gpsimd.dma_start(wkb[:], wk[D : D + 1, :])
            nc.gpsimd.dma_start(wvb[:], wv[D : D + 1, :])

            wo_sb = []
            for j in range(2):
                t = cpool.tile([128, D], FPR, tag=f"wo{j}")
                nc.gpsimd.dma_start(t[:], wo[j * 128 : (j + 1) * 128, :])
                wo_sb.append(t)

            qhT = rpool.tile([128, 2 * S], FPR, tag="qhT")
            khT = rpool.tile([128, 2 * S], FPR, tag="khT")
            vh = rpool.tile([128, 16 * 260], BF, tag="vh")
            attnT = rpool.tile([128, 2 * S], FPR, tag="attnT")
            sums = rpool.tile([16, 512], FPR, tag="sums")
            recip = rpool.tile([16, 512], FPR, tag="recip")

            # ---- phase 1: projections ------------------------------------
            # q/k: dt-outer with all 8 (j, sc) psums open, so one weight
            # load (lhsT) serves 4 s-chunk matmuls instead of reloading
            # the stationary operand every matmul.
            with tc.tile_pool(name="ps_proj", bufs=8, space="PSUM") as pp:
                for x_dram, w_sb, w_b, dst in (
                    (xq, wq_sb, wqb, qhT),
                    (xk, wk_sb, wkb, khT),
                ):
                    pss = {}
                    for j in range(2):
                        for sc in range(SC):
                            pss[j, sc] = pp.tile(
                                [128, 512], FP, tag="pp", name=f"pp_{j}_{sc}"
                            )
                    xt = {}
                    for dt in range(DT):
                        for sc in range(SC):
                            t = xpool.tile([128, 512], BF, tag="xin")
                            nc.sync.dma_start(
                                t[:],
                                x_dram[dt * 128 : (dt + 1) * 128, sc * 512 : (sc + 1) * 512],
                            )
                            xt[sc] = t
                        for j in range(2):
                            for sc in range(SC):
                                nc.tensor.matmul(
                                    pss[j, sc][:],
                                    w_sb[:, dt * 256 + j * 128 : dt * 256 + (j + 1) * 128],
                                    xt[sc][:],
                                    start=(dt == 0),
                                    stop=False,
                                )
                    for j in range(2):
                        for sc in range(SC):
                            nc.tensor.matmul(
                                pss[j, sc][:],
                                w_b[0:1, j * 128 : (j + 1) * 128],
                                ones[0:1, :],
                                start=False,
                                stop=True,
                            )
                            nc.vector.tensor_copy(
                                dst[:, j * S + sc * 512 : j * S + (sc + 1) * 512],
                                pss[j, sc][:],
                            )
            # v: vh' tiles [t=128, 260] per 128-key block
            with tc.tile_pool(name="ps_vproj", bufs=2, space="PSUM") as pv:
                for sc in range(SC):
                    xt = []
                    for dt in range(DT):
                        t = xpool.tile([128, 512], BF, tag="xin")
                        nc.sync.dma_start(
                            t[:],
                            xv[dt * 128 : (dt + 1) * 128, sc * 512 : (sc + 1) * 512],
                        )
                        xt.append(t)
                    for u in range(4):
                        tt = sc * 4 + u
                        ps = pv.tile([128, 260], FP, tag="ppv")
                        for dt in range(DT):
                            nc.tensor.matmul(
                                ps[:],
                                xt[dt][:, u * 128 : (u + 1) * 128],
                                wv_sb[:, dt * 260 : (dt + 1) * 260],
                                start=(dt == 0),
                                stop=False,
                            )
                        nc.tensor.matmul(
                            ps[:],
                            ones[0:1, 0:128],
                            wvb[0:1, :],
                            start=False,
                            stop=True,
                        )
                        nc.vector.tensor_copy(
                            vh[:, tt * 260 : (tt + 1) * 260], ps[:]
                        )

            # ---- phase 2: attention --------------------------------------
            with (
                tc.tile_pool(name="ps_sc", bufs=2, space="PSUM") as psc,
                tc.tile_pool(name="ps_pv", bufs=2, space="PSUM") as ppv,
            ):
                for hp in range(2):          # head pair
                    h0, h1 = 2 * hp, 2 * hp + 1
                    for sc in range(SC):     # 512 queries
                        pv0 = ppv.tile([EL, 512], FP, tag="pv0")
                        pv1 = ppv.tile([EL, 512], FP, tag="pv1")
                        exq = []

                        def scores(tt):
                            ps = psc.tile([128, 1024], FP, tag="sc")
                            nc.tensor.matmul(
                                ps[:, 0:512],
                                khT[0:64, hp * S + tt * 128 : hp * S + (tt + 1) * 128],
                                qhT[0:64, hp * S + sc * 512 : hp * S + (sc + 1) * 512],
                                start=True,
                                stop=True,
                            )
                            nc.tensor.matmul(
                                ps[:, 512:1024],
                                khT[64:128, hp * S + tt * 128 : hp * S + (tt + 1) * 128],
                                qhT[64:128, hp * S + sc * 512 : hp * S + (sc + 1) * 512],
                                start=True,
                                stop=True,
                            )
                            ex = epool.tile([128, 1024], BF, tag="exp")
                            nc.scalar.activation(ex[:], ps[:], EXP, scale=0.125)
                            exq.append(ex)

                        def pv(tt):
                            ex = exq[tt]
                            nc.tensor.matmul(
                                pv0[:],
                                vh[:, tt * 260 + (h0 % 4) * EL : tt * 260 + (h0 % 4) * EL + EL],
                                ex[:, 0:512],
                                start=(tt == 0),
                                stop=(tt == 15),
                            )
                            nc.tensor.matmul(
                                pv1[:],
                                vh[:, tt * 260 + (h1 % 4) * EL : tt * 260 + (h1 % 4) * EL + EL],
                                ex[:, 512:1024],
                                start=(tt == 0),
                                stop=(tt == 15),
                            )

                        scores(0)
                        for tt in range(1, 16):
                            scores(tt)
                            pv(tt - 1)
                        pv(15)

                        r0, r1 = h0 * SC + sc, h1 * SC + sc
                        st0 = spool.tile([EL, 512], FPR, tag="stage")
                        st1 = spool.tile([EL, 512], FPR, tag="stage")
                        nc.vector.tensor_copy(st0[:], pv0[:])
                        nc.vector.tensor_copy(st1[:], pv1[:])
                        nc.gpsimd.dma_start(
                            attnT[0:64, hp * S + sc * 512 : hp * S + (sc + 1) * 512],
                            st0[0:E, :],
                        )
                        nc.gpsimd.dma_start(
                            attnT[64:128, hp * S + sc * 512 : hp * S + (sc + 1) * 512],
                            st1[0:E, :],
                        )
                        nc.gpsimd.dma_start(sums[r0 : r0 + 1, :], st0[E : E + 1, :])
                        nc.gpsimd.dma_start(sums[r1 : r1 + 1, :], st1[E : E + 1, :])

            # ---- phase 3: normalize + output projection ------------------
            with nc.allow_low_precision(reason="fp32r recip feeds fp32r matmul"):
                nc.vector.reciprocal(recip[:], sums[:])
            with (
                tc.tile_pool(name="ps_rb", bufs=2, space="PSUM") as prb,
                tc.tile_pool(name="ps_op", bufs=4, space="PSUM") as pop,
            ):
                for j in range(2):
                    for sc in range(SC):
                        rb = prb.tile([128, 512], FP, tag="rb")
                        nc.tensor.matmul(
                            rb[:],
                            e_all[:, (j * 4 + sc) * 128 : (j * 4 + sc + 1) * 128],
                            recip[:],
                            start=True,
                            stop=True,
                        )
                        sl = attnT[:, j * S + sc * 512 : j * S + (sc + 1) * 512]
                        nc.vector.tensor_mul(sl, sl, rb[:])
                for st in range(16):
                    for nh in range(2):
                        ps = pop.tile([128, 512], FP, tag="op")
                        for j in range(2):
                            nc.tensor.matmul(
                                ps[:],
                                attnT[:, j * S + st * 128 : j * S + (st + 1) * 128],
                                wo_sb[j][:, nh * 512 : (nh + 1) * 512],
                                start=(j == 0),
                                stop=(j == 1),
                            )
                        ot = opool.tile([128, 512], FP, tag="outev")
                        if nh == 0:
                            nc.vector.tensor_copy(ot[:], ps[:])
                        else:
                            nc.scalar.copy(ot[:], ps[:])
                        nc.sync.dma_start(
                            out[st * 128 : (st + 1) * 128, nh * 512 : (nh + 1) * 512],
                            ot[:],
                        )

    nc.compile()
    return nc


def _get_nc():
    global _NC
    if _NC is None:
        _NC = _build()
    return _NC


def _in_maps(q, k, v, Wq, bq, Wk, bk, Wv, bv, Wo, bo):
    import ml_dtypes
    f32 = np.float32
    bf16 = ml_dtypes.bfloat16
    maps = []
    for c in range(N_CORES):
        b, hg = c // HG, c % HG
        hs = slice(hg * HG, (hg + 1) * HG)  # this core's 4 heads

        wq_h = np.zeros((D + 1, HG * E), f32)
        wq_h[:D] = np.transpose(Wq[hs], (1, 0, 2)).reshape(D, HG * E)
        wq_h[D] = bq[hs].reshape(-1)
        wk_h = np.zeros((D + 1, HG * E), f32)
        wk_h[:D] = np.transpose(Wk[hs], (1, 0, 2)).reshape(D, HG * E)
        wk_h[D] = bk[hs].reshape(-1)
        wv_h = np.zeros((D + 1, HG * EL), f32)
        for hl in range(HG):
            wv_h[:D, hl * EL : hl * EL + E] = Wv[hg * HG + hl]
            wv_h[D, hl * EL : hl * EL + E] = bv[hg * HG + hl]
            wv_h[D, hl * EL + E] = 1.0  # generates the ones column of vh'
        maps.append(
            {
                "xq": np.ascontiguousarray(q[b].T).astype(bf16),
                "xk": np.ascontiguousarray(k[b].T).astype(bf16),
                "xv": np.ascontiguousarray(v[b].T).astype(bf16),
                "wq": wq_h.astype(bf16),
                "wk": wk_h.astype(bf16),
                "wv": wv_h.astype(bf16),
                "wo": np.ascontiguousarray(
                    Wo[hg * HG * E : (hg + 1) * HG * E, :], dtype=f32
                ),
                "eall": _EALL,
                "ones": _ONES.astype(bf16),
            }
        )
    return maps


def _run(inputs, trace=False):
    from concourse.bass_utils import run_bass_kernel_spmd

    nc = _get_nc()
    maps = _in_maps(**inputs)
    res = run_bass_kernel_spmd(nc, maps, list(range(N_CORES)), trace=trace)
    bo = np.asarray(inputs["bo"], np.float32)
    out = np.zeros((B, S, D), np.float32)
    for b in range(B):
        acc = np.zeros((S, D), np.float32)
        for hg in range(HG):
            acc += res.results[b * HG + hg]["out_partial"]
        out[b] = acc + bo[None, :]
    return out, res.exec_time_ns


def kernel(**inputs):
    out, _ = _run(inputs, trace=False)
    return out


def kernel_traced(**inputs):
    return _run(inputs, trace=True)
